# revision 7
# baseline (speedup 1.0000x reference)
"""HMM forward-algorithm (nn_ForwardBackward) Trainium2 Bass kernel.

The reference is a strictly sequential scan of T=8192 steps, each a matvec of
the state row-vector with the SxS transition matrix followed by a rescale.
Rewritten in linear space with all per-step normalization folded into a
host-precomputed constant:

    u_t = (u_{t-1} @ A) * E_t * c_t,   c_t = 1/(0.5 * sum_j E_t[j])
    LL  = log(sum(u_{T-1})) + log(Z_0) - sum_t log(c_t)

c_t is a deterministic function of the inputs (it tracks the expected
per-step growth ~ S*mean(A)*mean-weighted(E)), so the bf16 state stays O(1)
(residual drift is a random walk of a few e^{+-5}, far within bf16 range)
and the device needs NO per-step Z / reciprocal / rescale: the entire LL
correction is a host-side f64 constant, and the device only computes
log(sum(u)) once at the end.

Tensor-parallel over 8 NeuronCores: core k owns a 256-column slice of A
(SBUF-resident, bf16).  Per step each core computes its 256 outputs with
weights-stationary bf16 matmuls (A blocks [128,128] stationary, state chunks
[128,1] moving, accumulated over the 16-chunk contraction into two PSUM
columns).  The PSUM->SBUF evacuation fused with the E-multiply is split
across DVE (column 0, tensor_tensor) and ACT (column 1, activation Copy with
per-partition scale AP) so the two run in parallel and every operand has
free_size 1.  Each core broadcasts its bf16 [128,2] slice (16B-strided into
the stationary state layout) to all 8 cores with SWDGE remote DMA (XOR-slot
addressing, shared per-parity arrival counters, runtime probe for the
logical->physical core permutation).  The steady-state critical path is
exactly three semaphore hops: PE -> {DVE,ACT} -> Pool(trigger) -> PE.
"""

import functools
import numpy as np
from ml_dtypes import bfloat16

from concourse import bass, bacc, mybir
from concourse.bass_utils import run_bass_kernel_spmd

S = 2048
T = 8192
NC = 8
SL = S // NC          # 256 state columns per core
CH = 16               # contraction chunks of 128
F = 2                 # payload columns per core (256 = 128*2)
EPS = 1e-8

f32 = mybir.dt.float32
bf16 = mybir.dt.bfloat16


def _jlocal():
    """local element j at (partition p, payload col c): j = 128*c + p."""
    p = np.arange(128)
    jl = np.zeros((128, F), dtype=np.int64)
    for c in range(F):
        jl[:, c] = 128 * c + p
    return jl


def _build_probe():
    """Tiny SPMD kernel: every core broadcasts its id-tile to all 8 XOR slots."""
    nc = bacc.Bacc(None, target_bir_lowering=False, num_devices=NC)
    x = nc.declare_dram_parameter("x", [128, 1], f32, isOutput=False)
    y = nc.declare_dram_parameter("y", [128, NC], f32, isOutput=True)
    with (
        nc.sbuf_tensor("S", [128, 1], f32) as Sb,
        nc.sbuf_tensor("R", [128, NC], f32) as Rb,
        nc.semaphore("dma_sem") as dma_sem,
        nc.semaphore("recv_sem") as recv_sem,
        nc.semaphore("send_local") as send_local,
        nc.semaphore("prep_sem") as prep_sem,
        nc.semaphore("out_sem") as out_sem,
        nc.Block() as block,
    ):
        @block.sync
        def _(sync):
            sync.dma_start(Sb[:, :], x[:, :]).then_inc(dma_sem, 16)

        @block.gpsimd
        def _(gp):
            for d in range(NC):
                rdests = [None] * NC
                rdests[d] = (0, d)
                gp.remote_dma_broadcast(
                    out_ap=Rb[:, d:d + 1], in_ap=Sb[:, :],
                    remote_sem=recv_sem, local_sem=send_local, rdests=rdests,
                ).then_inc(prep_sem, 1)
            gp.wait_ge(prep_sem, NC)
            gp.wait_ge(dma_sem, 16)
            gp.trigger_dma(count=NC)
            gp.wait_ge(recv_sem, 16)
            gp.dma_start(y[:, :], Rb[:, :]).then_inc(out_sem, 16)
            gp.wait_ge(out_sem, 16)
            gp.wait_ge(send_local, 16 * NC)
    nc.finalize()
    return nc


@functools.lru_cache(maxsize=1)
def _probe_sigma():
    """sigma[j][d] = logical id of the core whose slice lands in slot d of core j."""
    nc = _build_probe()
    ins = [{"x": np.full((128, 1), k, np.float32)} for k in range(NC)]
    res = run_bass_kernel_spmd(nc, ins, core_ids=list(range(NC)))
    sigma = []
    for j in range(NC):
        out = res.results[j]["y"]
        sigma.append(tuple(int(out[0, d]) for d in range(NC)))
    return tuple(sigma)


@functools.lru_cache(maxsize=8)
def _build_main(nsteps):
    """Full HMM forward kernel, unrolled over `nsteps` total steps (incl. t=0).

    Steps t = 1 .. nsteps-1 are compute+exchange rounds; then a Z-only round
    at t == nsteps reduces the final state and ACT takes its log.
    """
    Tm = nsteps
    nc = bacc.Bacc(None, target_bir_lowering=False, num_devices=NC)

    a_d = nc.declare_dram_parameter("a", [128, CH * 2 * 128], bf16, isOutput=False)
    e_d = nc.declare_dram_parameter("e", [128, F * Tm], f32, isOutput=False)
    r0_d = nc.declare_dram_parameter("r0", [128, 8 * CH], bf16, isOutput=False)
    out_d = nc.declare_dram_parameter("out", [1, 1], f32, isOutput=True)

    from contextlib import ExitStack
    with ExitStack() as es:
        ec = es.enter_context
        Abuf = ec(nc.sbuf_tensor("Abuf", [128, CH * 2 * 128], bf16))
        Ebuf = ec(nc.sbuf_tensor("Ebuf", [128, F * Tm], f32))
        # R layout: slot d = cols [16d, 16d+16); chunk c = col 8c ([128,1] bf16)
        R = [ec(nc.sbuf_tensor(f"R{i}", [128, 8 * CH], bf16)) for i in range(2)]
        # Sb: one 32-byte contiguous send block; payload at cols 0 and 8
        Sb = [ec(nc.sbuf_tensor(f"S{i}", [128, 16], bf16)) for i in range(2)]
        ones = ec(nc.sbuf_tensor("ones", [128, 128], bf16))
        LLacc = ec(nc.sbuf_tensor("LLacc", [1, 1], f32))
        P = [[ec(nc.psum_tensor(f"P{i}_{h}", [128, 512], f32)) for h in range(2)] for i in range(2)]
        Zp = ec(nc.psum_tensor("Zp", [128, 512], f32))
        sem = lambda n: ec(nc.semaphore(n))
        ld_sem = sem("ld_sem"); exp_sem = sem("exp_sem"); misc_sem = sem("misc_sem")
        mmh_sem = sem("mmh_sem"); mm_sem = sem("mm_sem"); sa_sem = sem("sa_sem")
        prep_sem = sem("prep_sem"); sloc_sem = sem("sloc_sem")
        zmm_sem = sem("zmm_sem"); fin_sem = sem("fin_sem"); out_sem = sem("out_sem")
        # shared arrival counters, split by round parity: after round r lands,
        # rv[r%2] has value 16 * ceil(r/2).
        rv = [sem(f"rv{p}") for p in range(2)]
        block = ec(nc.Block())
        EXP_CHUNK = 1024
        n_exp = (F * Tm + EXP_CHUNK - 1) // EXP_CHUNK

        @block.sync
        def _(sync):
            sync.dma_start(Abuf[:, :], a_d[:, :]).then_inc(ld_sem, 16)
            sync.dma_start(Ebuf[:, :], e_d[:, :]).then_inc(ld_sem, 16)
            sync.dma_start(R[1][:, :], r0_d[:, :]).then_inc(ld_sem, 16)

        @block.scalar
        def _(act):
            act.wait_ge(ld_sem, 48)
            for i in range(n_exp):
                lo = i * EXP_CHUNK
                hi = min(lo + EXP_CHUNK, F * Tm)
                act.activation(
                    Ebuf[:, lo:hi], Ebuf[:, lo:hi],
                    mybir.ActivationFunctionType.Exp,
                ).then_inc(exp_sem, 1)
            act.wait_ge(exp_sem, n_exp)
            act.wait_ge(misc_sem, 2)
            for t in range(1, Tm):
                bt = t % 2
                if t >= 3:
                    act.wait_ge(sloc_sem, 16 * NC * (t - 2))
                act.wait_ge(mm_sem, t)
                act.activation(
                    Sb[bt][:, 8:9], P[bt][1][:, 0:1],
                    mybir.ActivationFunctionType.Copy,
                    scale=Ebuf[:, F * t + 1:F * t + 2],
                ).then_inc(sa_sem, 1)
            # epilogue: LL = ln(Z) of the final reduced state
            act.wait_ge(zmm_sem, 1)
            act.activation(
                LLacc[0:1, 0:1], Zp[0:1, 0:1], mybir.ActivationFunctionType.Ln,
            ).then_inc(fin_sem, 1)

        @block.tensor
        def _(te):
            te.wait_ge(ld_sem, 48)
            for t in range(1, Tm):
                bt = t % 2
                if t >= 2:
                    te.wait_ge(rv[(t - 1) % 2], 16 * (t // 2))
                if t >= 3:
                    te.wait_ge(sa_sem, 2 * (t - 2))
                lasth = [None, None]
                for c in range(CH):
                    for h in range(2):
                        lasth[h] = nc.tensor.matmul(
                            P[bt][h][:, 0:1],
                            Abuf[:, (c * 2 + h) * 128:(c * 2 + h + 1) * 128],
                            R[bt][:, 8 * c:8 * c + 1],
                            start=(c == 0), stop=(c == CH - 1),
                        )
                lasth[0].then_inc(mmh_sem, 1)
                lasth[1].then_inc(mm_sem, 1)
            # Z-only round on the final broadcast state
            te.wait_ge(rv[(Tm - 1) % 2], 16 * (Tm // 2))
            te.wait_ge(misc_sem, 3)
            lastz = None
            for c in range(CH):
                lastz = nc.tensor.matmul(
                    Zp[:, 0:1], ones[:, 0:128], R[Tm % 2][:, 8 * c:8 * c + 1],
                    start=(c == 0), stop=(c == CH - 1),
                )
            lastz.then_inc(zmm_sem, 1)

        @block.vector
        def _(ve):
            ve.memset(Sb[0][:, :], 0.0).then_inc(misc_sem, 1)
            ve.memset(Sb[1][:, :], 0.0).then_inc(misc_sem, 1)
            ve.memset(ones[:, :], 1.0).then_inc(misc_sem, 1)
            ve.drain()
            ve.wait_ge(exp_sem, n_exp)
            for t in range(1, Tm):
                bt = t % 2
                if t >= 3:
                    ve.wait_ge(sloc_sem, 16 * NC * (t - 2))
                ve.wait_ge(mmh_sem, t)
                ve.tensor_tensor(
                    Sb[bt][:, 0:1], P[bt][0][:, 0:1], Ebuf[:, F * t:F * t + 1],
                    op=mybir.AluOpType.mult,
                ).then_inc(sa_sem, 1)

        @block.gpsimd
        def _(gp):
            for t in range(1, Tm):
                bn = (t + 1) % 2
                if t >= 2:
                    gp.wait_ge(sloc_sem, 16 * NC * (t - 1))
                for d in range(NC):
                    rdests = [None] * NC
                    rdests[d] = (0, d)
                    gp.remote_dma_broadcast(
                        out_ap=R[bn][:, 16 * d:16 * (d + 1)],
                        in_ap=Sb[t % 2][:, :],
                        remote_sem=rv[t % 2], local_sem=sloc_sem, rdests=rdests,
                    ).then_inc(prep_sem, 1)
                gp.wait_ge(prep_sem, NC * t)
                gp.wait_ge(sa_sem, 2 * t)
                gp.trigger_dma(count=NC)
            # epilogue: write out LL
            gp.wait_ge(fin_sem, 1)
            gp.dma_start(out_d[:, :], LLacc[0:1, 0:1]).then_inc(out_sem, 16)
            gp.wait_ge(out_sem, 16)
            gp.wait_ge(sloc_sem, 16 * NC * (Tm - 1))

    nc.finalize()
    return nc


def _prep_inputs(probt, transition, pi, sigma, nsteps):
    """Per-core rearranged input arrays plus the host-side LL constant."""
    jl = _jlocal()
    p = np.arange(128)
    probt64 = probt.astype(np.float64)
    # per-step growth compensation: c_t = 1 / (0.5 * sum_j exp(probt[t,j]))
    m = probt64.max(axis=1)
    lse = m + np.log(np.exp(probt64 - m[:, None]).sum(axis=1))
    lnc = -(lse + np.log(0.5))                     # [T] f64; lnc[0] unused
    E0 = np.exp(probt64[0])
    s0 = np.clip(pi.astype(np.float64), EPS, None) * E0
    Z0 = s0.sum()
    u0 = (s0 / Z0).astype(np.float32)
    host_const = np.log(Z0) - lnc[1:nsteps].sum()

    e_shift = (probt64[:nsteps] + lnc[:nsteps, None]).astype(np.float32)

    in_maps = []
    for k in range(NC):
        rows = np.zeros(S, dtype=np.int64)
        for ch in range(CH):
            snd = sigma[k][ch // 2]
            rows[ch * 128 + p] = SL * snd + jl[:, ch % 2]
        # A slice as stationary blocks: Abuf[p, (c*2+h)*128 + m] =
        # A[row(c, p), SL*k + 128h + m]
        Ak = transition[rows][:, SL * k:SL * (k + 1)].astype(np.float32)
        Abuf = (
            Ak.reshape(CH, 128, 2, 128)
            .transpose(1, 0, 2, 3)
            .reshape(128, CH * 2 * 128)
        ).astype(bfloat16)
        cols = SL * k + jl
        Ek = e_shift[:, cols]                      # [Tm, 128, 2]
        Ebuf = np.ascontiguousarray(
            Ek.transpose(1, 0, 2).reshape(128, F * nsteps)
        ).astype(np.float32)
        r0 = np.zeros((128, 8 * CH), dtype=bfloat16)
        r0[:, 0:8 * CH:8] = u0[rows].reshape(CH, 128).T.astype(bfloat16)
        in_maps.append({"a": np.ascontiguousarray(Abuf), "e": Ebuf, "r0": r0})
    return in_maps, host_const


def _run(probt, transition, pi, nsteps):
    sigma = _probe_sigma()
    nc = _build_main(nsteps)
    in_maps, host_const = _prep_inputs(probt, transition, pi, sigma, nsteps)
    res = run_bass_kernel_spmd(nc, in_maps, core_ids=list(range(NC)))
    return np.float32(np.float64(res.results[0]["out"][0, 0]) + host_const)


def kernel(probt, transition, pi):
    ll = _run(np.asarray(probt), np.asarray(transition), np.asarray(pi), T)
    return np.float32(ll)
